# revision 12
# baseline (speedup 1.0000x reference)
"""Trainium2 Bass kernel: masked (sparse-adjacency) attention.

Computes, for full inputs:
    adj    = adjs[idx]                      # [Na, N] bool
    scores = (anchor @ wt) @ x.T            # [Na, N]
    atten  = softmax(where(adj, scores, -inf) / T, axis=1)
    out    = weight[idx] * (atten @ x)      # [Na, d_out]

Sharding: anchors (rows of the score matrix) are split across the 8
NeuronCores, 1280 rows per core (Na=10000 padded to 10240). x / wt are
replicated; the adjacency is shipped pre-transposed per shard.

Per-core device pipeline (all j-tiles of 128 x-rows, N=10000 padded to
10112 = 79*128):
  S^T[j,a] = X^T.T @ Q^T          (PE, fp32; Q^T = wt.T @ anchor.T)
  P^T      = exp(S^T / T)         (ACT, PSUM -> SBUF, bf16 out)
  PM^T     = P^T * adjT           (DVE tensor_tensor, bf16 2x mode)
  O^T     += [X | 1].T @ PM^T     (PE accumulating over j; the ones
                                   column yields softmax denominators)
Tail: PE-transpose O^T back to [a, 65], scale rows by
weight[idx] / denom, DMA out.

Masking happens *after* exp as a multiply by {0,1}: exp(s/T) is computed
for every entry (bounded: |s| < ~2 for this data, exp arg < ~30), and
multiplying by the adjacency bit zeroes masked entries exactly, which is
mathematically identical to softmax over the masked entries.
"""

import numpy as np
import ml_dtypes

import concourse.bacc as bacc
import concourse.bass as bass
import concourse.mybir as mybir
import concourse.tile as tile
from concourse.bass_utils import run_bass_kernel_spmd

F32 = mybir.dt.float32
BF16 = mybir.dt.bfloat16

N_CORES = 8
N = 10000          # x rows (softmax width)
NA = 10000         # anchors
D_IN = 256
D_OUT = 64
TEMP = 0.07

NJ_TILES = 79                 # ceil(10000 / 128)
NJ = NJ_TILES * 128           # 10112, padded x-rows
A_CORE = 1280                 # anchors per core (10240 padded / 8)
A_CHUNK = 640                 # anchor columns processed per pass
N_ACHUNK = A_CORE // A_CHUNK  # 2
M_AUG = D_OUT + 1             # 65: d_out columns + ones column


def _build_bass():
    # Bacc (not plain Bass): its compile() runs the wait-splitting passes
    # (move_matmul_waits_to_ldweights / generate_event_semaphores) that
    # keep every instruction within the TRN2 1-sync-wait ISA limit.
    nc = bacc.Bacc(
        "TRN2",
        target_bir_lowering=False,
        debug=False,
        num_devices=N_CORES,
    )
    xT = nc.dram_tensor("xT", [D_OUT, NJ], F32, kind="ExternalInput").ap()
    xaug = nc.dram_tensor(
        "xaug", [128, NJ_TILES * M_AUG], BF16, kind="ExternalInput"
    ).ap()
    anchT = nc.dram_tensor("anchT", [D_IN, A_CORE], F32, kind="ExternalInput").ap()
    adjT = nc.dram_tensor("adjT", [NJ, A_CORE], BF16, kind="ExternalInput").ap()
    wt = nc.dram_tensor("wt", [D_IN, D_OUT], F32, kind="ExternalInput").ap()
    wscale = nc.dram_tensor("wscale", [128, 1], F32, kind="ExternalInput").ap()
    ident = nc.dram_tensor("ident", [128, 128], F32, kind="ExternalInput").ap()
    out = nc.dram_tensor("out", [A_CORE, D_OUT], F32, kind="ExternalOutput").ap()

    EXP = mybir.ActivationFunctionType.Exp

    with tile.TileContext(nc) as tc:
        with tc.tile_pool(name="const", bufs=1) as const:
            xT_sb = const.tile([D_OUT, NJ], F32)
            nc.sync.dma_start(xT_sb[:], xT[:])
            xaug_sb = const.tile([128, NJ_TILES * M_AUG], BF16)
            nc.sync.dma_start(xaug_sb[:], xaug[:])
            ident_sb = const.tile([128, 128], F32)
            nc.sync.dma_start(ident_sb[:], ident[:])
            wscale_sb = const.tile([128, 1], F32)
            nc.sync.dma_start(wscale_sb[:], wscale[:])
            qt_sb = const.tile([D_OUT, A_CORE], F32)
            ot_sb = const.tile([M_AUG, A_CORE], F32)

            # ---- Q^T = wt.T @ anchor.T  -> [64, 1280] ----
            with (
                tc.tile_pool(name="pre", bufs=1) as pre,
                tc.tile_pool(name="prepsum", bufs=1, space="PSUM") as prepsum,
            ):
                an0 = pre.tile([128, A_CORE], F32)
                nc.sync.dma_start(an0[:], anchT[0:128, :])
                an1 = pre.tile([128, A_CORE], F32)
                nc.sync.dma_start(an1[:], anchT[128:256, :])
                wt0 = pre.tile([128, D_OUT], F32)
                nc.sync.dma_start(wt0[:], wt[0:128, :])
                wt1 = pre.tile([128, D_OUT], F32)
                nc.sync.dma_start(wt1[:], wt[128:256, :])
                qt_ps = prepsum.tile([D_OUT, A_CORE], F32)
                for n0 in range(0, A_CORE, 512):
                    nw = min(512, A_CORE - n0)
                    nc.tensor.matmul(
                        qt_ps[:, n0 : n0 + nw],
                        wt0[:],
                        an0[:, n0 : n0 + nw],
                        start=True,
                        stop=False,
                    )
                    nc.tensor.matmul(
                        qt_ps[:, n0 : n0 + nw],
                        wt1[:],
                        an1[:, n0 : n0 + nw],
                        start=False,
                        stop=True,
                    )
                nc.vector.tensor_copy(qt_sb[:], qt_ps[:])

            # ---- main loop over anchor chunks / j tiles ----
            with (
                tc.tile_pool(name="adjp", bufs=6) as adjp,
                tc.tile_pool(name="pp", bufs=3) as pp,
                tc.tile_pool(name="pmp", bufs=3) as pmp,
                tc.tile_pool(name="spsum", bufs=3, space="PSUM") as spsum,
                tc.tile_pool(name="opsum", bufs=1, space="PSUM") as opsum,
            ):
                for ai in range(N_ACHUNK):
                    a0 = ai * A_CHUNK
                    o_ps = opsum.tile([M_AUG, A_CHUNK], F32)
                    for j in range(NJ_TILES):
                        xt_w = xT_sb[:, j * 128 : (j + 1) * 128]
                        s_ps = spsum.tile([128, A_CHUNK], F32)
                        nc.tensor.matmul(
                            s_ps[:, 0:512],
                            xt_w,
                            qt_sb[:, a0 : a0 + 512],
                            start=True,
                            stop=True,
                        )
                        nc.tensor.matmul(
                            s_ps[:, 512:640],
                            xt_w,
                            qt_sb[:, a0 + 512 : a0 + 640],
                            start=True,
                            stop=True,
                        )
                        adj_t = adjp.tile([128, A_CHUNK], BF16)
                        nc.sync.dma_start(
                            adj_t[:], adjT[j * 128 : (j + 1) * 128, a0 : a0 + A_CHUNK]
                        )
                        p_t = pp.tile([128, A_CHUNK], BF16)
                        nc.scalar.activation(p_t[:], s_ps[:], EXP, scale=1.0 / TEMP)
                        pm_t = pmp.tile([128, A_CHUNK], BF16)
                        nc.vector.tensor_mul(pm_t[:], p_t[:], adj_t[:])
                        xa_w = xaug_sb[:, j * M_AUG : (j + 1) * M_AUG]
                        nc.tensor.matmul(
                            o_ps[:, 0:512],
                            xa_w,
                            pm_t[:, 0:512],
                            start=(j == 0),
                            stop=(j == NJ_TILES - 1),
                        )
                        nc.tensor.matmul(
                            o_ps[:, 512:640],
                            xa_w,
                            pm_t[:, 512:640],
                            start=(j == 0),
                            stop=(j == NJ_TILES - 1),
                        )
                    nc.vector.tensor_copy(ot_sb[:, a0 : a0 + A_CHUNK], o_ps[:])

            # ---- tail: transpose back, normalize, scale, store ----
            with (
                tc.tile_pool(name="tpsum", bufs=2, space="PSUM") as tpsum,
                tc.tile_pool(name="tail", bufs=2) as tail,
            ):
                for k in range(A_CORE // 128):
                    t_ps = tpsum.tile([128, M_AUG], F32)
                    nc.tensor.transpose(
                        t_ps[:],
                        ot_sb[0:M_AUG, k * 128 : (k + 1) * 128],
                        ident_sb[0:M_AUG, 0:M_AUG],
                    )
                    rec = tail.tile([128, 1], F32)
                    nc.vector.reciprocal(rec[:], t_ps[:, D_OUT : D_OUT + 1])
                    rec2 = tail.tile([128, 1], F32)
                    nc.vector.tensor_mul(rec2[:], rec[:], wscale_sb[:])
                    o_t = tail.tile([128, D_OUT], F32)
                    nc.vector.tensor_scalar_mul(o_t[:], t_ps[:, 0:D_OUT], rec2[:])
                    nc.sync.dma_start(out[k * 128 : (k + 1) * 128, :], o_t[:])

    nc.compile()
    return nc


def _prep_inputs(x, weight, adjs, idx, anchor, wt):
    i = int(np.asarray(idx))
    x = np.asarray(x, dtype=np.float32)
    anchor = np.asarray(anchor, dtype=np.float32)
    wt = np.asarray(wt, dtype=np.float32)
    adj = np.asarray(adjs)[i]  # [Na, N] bool
    w = float(np.asarray(weight)[i])

    NAP = N_CORES * A_CORE  # 10240

    xT = np.zeros((D_OUT, NJ), dtype=np.float32)
    xT[:, :N] = x.T

    xaug = np.zeros((NJ, M_AUG), dtype=ml_dtypes.bfloat16)
    xaug[:N, :D_OUT] = x
    xaug[:N, D_OUT] = 1.0
    xaug_strip = np.ascontiguousarray(
        xaug.reshape(NJ_TILES, 128, M_AUG).transpose(1, 0, 2).reshape(128, -1)
    )

    anchorT = np.zeros((D_IN, NAP), dtype=np.float32)
    anchorT[:, :NA] = anchor.T

    # adjacency, transposed to [N, Na], as bf16 {0.0, 1.0}
    adj_u16 = np.zeros((NJ, NAP), dtype=np.uint16)
    adj_u16[:N, :NA] = adj.T
    adj_u16 *= 0x3F80  # bf16 bit pattern of 1.0
    # padded anchor columns: one fake edge to x-row 0 so denominators are
    # finite (those rows are discarded on the host)
    adj_u16[0, NA:] = 0x3F80
    adj_bf = adj_u16.view(ml_dtypes.bfloat16)

    ident = np.eye(128, dtype=np.float32)
    wscale = np.full((128, 1), w, dtype=np.float32)

    in_maps = []
    for c in range(N_CORES):
        sl = slice(c * A_CORE, (c + 1) * A_CORE)
        in_maps.append(
            {
                "xT": xT,
                "xaug": xaug_strip,
                "anchT": np.ascontiguousarray(anchorT[:, sl]),
                "adjT": np.ascontiguousarray(adj_bf[:, sl]),
                "wt": wt,
                "wscale": wscale,
                "ident": ident,
            }
        )
    return in_maps


def run(x, weight, adjs, idx, anchor, wt, trace=False, **spmd_kwargs):
    in_maps = _prep_inputs(x, weight, adjs, idx, anchor, wt)
    nc = _build_bass()
    res = run_bass_kernel_spmd(
        nc, in_maps, core_ids=list(range(N_CORES)), trace=trace, **spmd_kwargs
    )
    out = np.concatenate([r["out"] for r in res.results], axis=0)[:NA]
    return np.ascontiguousarray(out.astype(np.float32)), res


def kernel(x, weight, adjs, idx, anchor, wt):
    out, _ = run(x, weight, adjs, idx, anchor, wt)
    return out


# revision 16
# speedup vs baseline: 1.9021x; 1.9021x over previous
"""Trainium2 Bass kernel: masked (sparse-adjacency) attention.

Computes, for full inputs:
    adj    = adjs[idx]                      # [Na, N] bool
    scores = (anchor @ wt) @ x.T            # [Na, N]
    atten  = softmax(where(adj, scores, -inf) / T, axis=1)
    out    = weight[idx] * (atten @ x)      # [Na, d_out]

Sharding: anchors (rows of the score matrix) are split across the 8
NeuronCores, 1280 rows per core (Na=10000 padded to 10240). x / wt are
replicated; the adjacency is shipped pre-transposed per shard.

Per-core device pipeline (all j-tiles of 128 x-rows, N=10000 padded to
10112 = 79*128):
  S^T[j,a] = X^T.T @ Q^T          (PE, fp32; Q^T = wt.T @ anchor.T)
  P^T      = exp(S^T / T)         (ACT, PSUM -> SBUF, bf16 out)
  PM^T     = P^T * adjT           (DVE tensor_tensor, bf16 2x mode)
  O^T     += [X | 1].T @ PM^T     (PE accumulating over j; the ones
                                   column yields softmax denominators)
Tail: PE-transpose O^T back to [a, 65], scale rows by
weight[idx] / denom, DMA out.

Masking happens *after* exp as a multiply by {0,1}: exp(s/T) is computed
for every entry (bounded: |s| < ~2 for this data, exp arg < ~30), and
multiplying by the adjacency bit zeroes masked entries exactly, which is
mathematically identical to softmax over the masked entries.
"""

import numpy as np
import ml_dtypes

import concourse.bacc as bacc
import concourse.bass as bass
import concourse.mybir as mybir
import concourse.tile as tile
from concourse.bass_utils import run_bass_kernel_spmd

F32 = mybir.dt.float32
F32R = mybir.dt.float32r  # fp32 fast-path: 1 PE cycle/row at N>=256 (vs 4 for fp32)
BF16 = mybir.dt.bfloat16

N_CORES = 8
N = 10000          # x rows (softmax width)
NA = 10000         # anchors
D_IN = 256
D_OUT = 64
TEMP = 0.07

NJ_TILES = 79                 # ceil(10000 / 128)
NJ = NJ_TILES * 128           # 10112, padded x-rows
A_CORE = 1280                 # anchors per core (10240 padded / 8)
A_CHUNK = 640                 # anchor columns processed per pass
N_ACHUNK = A_CORE // A_CHUNK  # 2
M_AUG = D_OUT + 1             # 65: d_out columns + ones column


def _build_bass():
    # Bacc (not plain Bass): its compile() runs the wait-splitting passes
    # (move_matmul_waits_to_ldweights / generate_event_semaphores) that
    # keep every instruction within the TRN2 1-sync-wait ISA limit.
    nc = bacc.Bacc(
        "TRN2",
        target_bir_lowering=False,
        debug=False,
        num_devices=N_CORES,
    )
    xT = nc.dram_tensor("xT", [D_OUT, NJ], F32R, kind="ExternalInput").ap()
    xaug = nc.dram_tensor(
        "xaug", [128, NJ_TILES * M_AUG], BF16, kind="ExternalInput"
    ).ap()
    anchT = nc.dram_tensor("anchT", [D_IN, A_CORE], F32R, kind="ExternalInput").ap()
    adjT = nc.dram_tensor("adjT", [NJ, A_CORE], BF16, kind="ExternalInput").ap()
    wt = nc.dram_tensor("wt", [D_IN, D_OUT], F32R, kind="ExternalInput").ap()
    wscale = nc.dram_tensor("wscale", [128, 1], F32, kind="ExternalInput").ap()
    ident = nc.dram_tensor("ident", [128, 128], F32, kind="ExternalInput").ap()
    out = nc.dram_tensor("out", [A_CORE, D_OUT], F32, kind="ExternalOutput").ap()

    EXP = mybir.ActivationFunctionType.Exp

    with tile.TileContext(nc) as tc:
        with tc.tile_pool(name="const", bufs=1) as const:
            xT_sb = const.tile([D_OUT, NJ], F32R)
            nc.sync.dma_start(xT_sb[:], xT[:])
            xaug_sb = const.tile([128, NJ_TILES * M_AUG], BF16)
            nc.sync.dma_start(xaug_sb[:], xaug[:])
            ident_sb = const.tile([128, 128], F32)
            nc.sync.dma_start(ident_sb[:], ident[:])
            wscale_sb = const.tile([128, 1], F32)
            nc.sync.dma_start(wscale_sb[:], wscale[:])
            qt_sb = const.tile([D_OUT, A_CORE], F32R)
            ot_sb = const.tile([M_AUG, A_CORE], F32)

            # ---- Q^T = wt.T @ anchor.T  -> [64, 1280] ----
            with (
                tc.tile_pool(name="pre", bufs=1) as pre,
                tc.tile_pool(name="prepsum", bufs=1, space="PSUM") as prepsum,
            ):
                an0 = pre.tile([128, A_CORE], F32R)
                nc.sync.dma_start(an0[:], anchT[0:128, :])
                an1 = pre.tile([128, A_CORE], F32R)
                nc.sync.dma_start(an1[:], anchT[128:256, :])
                wt0 = pre.tile([128, D_OUT], F32R)
                nc.sync.dma_start(wt0[:], wt[0:128, :])
                wt1 = pre.tile([128, D_OUT], F32R)
                nc.sync.dma_start(wt1[:], wt[128:256, :])
                qt_ps = prepsum.tile([D_OUT, A_CORE], F32)
                for n0 in range(0, A_CORE, 512):
                    nw = min(512, A_CORE - n0)
                    nc.tensor.matmul(
                        qt_ps[:, n0 : n0 + nw],
                        wt0[:],
                        an0[:, n0 : n0 + nw],
                        start=True,
                        stop=False,
                    )
                    nc.tensor.matmul(
                        qt_ps[:, n0 : n0 + nw],
                        wt1[:],
                        an1[:, n0 : n0 + nw],
                        start=False,
                        stop=True,
                    )
                nc.vector.tensor_copy(qt_sb[:], qt_ps[:])

            # ---- main loop over anchor chunks / j tiles ----
            with (
                tc.tile_pool(name="adjp", bufs=6) as adjp,
                tc.tile_pool(name="pp", bufs=3) as pp,
                tc.tile_pool(name="pmp", bufs=3) as pmp,
                tc.tile_pool(name="spsum", bufs=3, space="PSUM") as spsum,
                tc.tile_pool(name="opsum", bufs=1, space="PSUM") as opsum,
            ):
                for ai in range(N_ACHUNK):
                    a0 = ai * A_CHUNK
                    o_ps = opsum.tile([M_AUG, A_CHUNK], F32)
                    for j in range(NJ_TILES):
                        xt_w = xT_sb[:, j * 128 : (j + 1) * 128]
                        s_ps = spsum.tile([128, A_CHUNK], F32)
                        nc.tensor.matmul(
                            s_ps[:, 0:512],
                            xt_w,
                            qt_sb[:, a0 : a0 + 512],
                            start=True,
                            stop=True,
                        )
                        nc.tensor.matmul(
                            s_ps[:, 512:640],
                            xt_w,
                            qt_sb[:, a0 + 512 : a0 + 640],
                            start=True,
                            stop=True,
                        )
                        adj_t = adjp.tile([128, A_CHUNK], BF16)
                        nc.sync.dma_start(
                            adj_t[:], adjT[j * 128 : (j + 1) * 128, a0 : a0 + A_CHUNK]
                        )
                        p_t = pp.tile([128, A_CHUNK], BF16)
                        nc.scalar.activation(p_t[:], s_ps[:], EXP, scale=1.0 / TEMP)
                        pm_t = pmp.tile([128, A_CHUNK], BF16)
                        nc.vector.tensor_mul(pm_t[:], p_t[:], adj_t[:])
                        xa_w = xaug_sb[:, j * M_AUG : (j + 1) * M_AUG]
                        nc.tensor.matmul(
                            o_ps[:, 0:512],
                            xa_w,
                            pm_t[:, 0:512],
                            start=(j == 0),
                            stop=(j == NJ_TILES - 1),
                        )
                        nc.tensor.matmul(
                            o_ps[:, 512:640],
                            xa_w,
                            pm_t[:, 512:640],
                            start=(j == 0),
                            stop=(j == NJ_TILES - 1),
                        )
                    nc.vector.tensor_copy(ot_sb[:, a0 : a0 + A_CHUNK], o_ps[:])

            # ---- tail: transpose back, normalize, scale, store ----
            with (
                tc.tile_pool(name="tpsum", bufs=2, space="PSUM") as tpsum,
                tc.tile_pool(name="tail", bufs=2) as tail,
            ):
                for k in range(A_CORE // 128):
                    t_ps = tpsum.tile([128, M_AUG], F32)
                    nc.tensor.transpose(
                        t_ps[:],
                        ot_sb[0:M_AUG, k * 128 : (k + 1) * 128],
                        ident_sb[0:M_AUG, 0:M_AUG],
                    )
                    rec = tail.tile([128, 1], F32)
                    nc.vector.reciprocal(rec[:], t_ps[:, D_OUT : D_OUT + 1])
                    rec2 = tail.tile([128, 1], F32)
                    nc.vector.tensor_mul(rec2[:], rec[:], wscale_sb[:])
                    o_t = tail.tile([128, D_OUT], F32)
                    nc.vector.tensor_scalar_mul(o_t[:], t_ps[:, 0:D_OUT], rec2[:])
                    nc.sync.dma_start(out[k * 128 : (k + 1) * 128, :], o_t[:])

    nc.compile()
    return nc


def _prep_inputs(x, weight, adjs, idx, anchor, wt):
    i = int(np.asarray(idx))
    x = np.asarray(x, dtype=np.float32)
    anchor = np.asarray(anchor, dtype=np.float32)
    wt = np.asarray(wt, dtype=np.float32)
    adj = np.asarray(adjs)[i]  # [Na, N] bool
    w = float(np.asarray(weight)[i])

    NAP = N_CORES * A_CORE  # 10240

    xT = np.zeros((D_OUT, NJ), dtype=np.float32)
    xT[:, :N] = x.T

    xaug = np.zeros((NJ, M_AUG), dtype=ml_dtypes.bfloat16)
    xaug[:N, :D_OUT] = x
    xaug[:N, D_OUT] = 1.0
    xaug_strip = np.ascontiguousarray(
        xaug.reshape(NJ_TILES, 128, M_AUG).transpose(1, 0, 2).reshape(128, -1)
    )

    anchorT = np.zeros((D_IN, NAP), dtype=np.float32)
    anchorT[:, :NA] = anchor.T

    # adjacency, transposed to [N, Na], as bf16 {0.0, 1.0}
    adj_u16 = np.zeros((NJ, NAP), dtype=np.uint16)
    adj_u16[:N, :NA] = adj.T
    adj_u16 *= 0x3F80  # bf16 bit pattern of 1.0
    # padded anchor columns: one fake edge to x-row 0 so denominators are
    # finite (those rows are discarded on the host)
    adj_u16[0, NA:] = 0x3F80
    adj_bf = adj_u16.view(ml_dtypes.bfloat16)

    ident = np.eye(128, dtype=np.float32)
    wscale = np.full((128, 1), w, dtype=np.float32)

    in_maps = []
    for c in range(N_CORES):
        sl = slice(c * A_CORE, (c + 1) * A_CORE)
        in_maps.append(
            {
                "xT": xT,
                "xaug": xaug_strip,
                "anchT": np.ascontiguousarray(anchorT[:, sl]),
                "adjT": np.ascontiguousarray(adj_bf[:, sl]),
                "wt": wt,
                "wscale": wscale,
                "ident": ident,
            }
        )
    return in_maps


def run(x, weight, adjs, idx, anchor, wt, trace=False, **spmd_kwargs):
    in_maps = _prep_inputs(x, weight, adjs, idx, anchor, wt)
    nc = _build_bass()
    res = run_bass_kernel_spmd(
        nc, in_maps, core_ids=list(range(N_CORES)), trace=trace, **spmd_kwargs
    )
    out = np.concatenate([r["out"] for r in res.results], axis=0)[:NA]
    return np.ascontiguousarray(out.astype(np.float32)), res


def kernel(x, weight, adjs, idx, anchor, wt):
    out, _ = run(x, weight, adjs, idx, anchor, wt)
    return out


# revision 19
# speedup vs baseline: 1.9177x; 1.0082x over previous
"""Trainium2 Bass kernel: masked (sparse-adjacency) attention.

Computes, for full inputs:
    adj    = adjs[idx]                      # [Na, N] bool
    scores = (anchor @ wt) @ x.T            # [Na, N]
    atten  = softmax(where(adj, scores, -inf) / T, axis=1)
    out    = weight[idx] * (atten @ x)      # [Na, d_out]

Sharding: anchors (rows of the score matrix) are split across the 8
NeuronCores, 1280 rows per core (Na=10000 padded to 10240). x / wt are
replicated; the adjacency is shipped pre-transposed per shard.

Per-core device pipeline (all j-tiles of 128 x-rows, N=10000 padded to
10112 = 79*128):
  S^T[j,a] = X^T.T @ Q^T          (PE, fp32; Q^T = wt.T @ anchor.T)
  P^T      = exp(S^T / T)         (ACT, PSUM -> SBUF, bf16 out)
  PM^T     = P^T * adjT           (DVE tensor_tensor, bf16 2x mode)
  O^T     += [X | 1].T @ PM^T     (PE accumulating over j; the ones
                                   column yields softmax denominators)
Tail: PE-transpose O^T back to [a, 65], scale rows by
weight[idx] / denom, DMA out.

Masking happens *after* exp as a multiply by {0,1}: exp(s/T) is computed
for every entry (bounded: |s| < ~2 for this data, exp arg < ~30), and
multiplying by the adjacency bit zeroes masked entries exactly, which is
mathematically identical to softmax over the masked entries.
"""

import numpy as np
import ml_dtypes

import concourse.bacc as bacc
import concourse.bass as bass
import concourse.mybir as mybir
import concourse.tile as tile
from concourse.bass_utils import run_bass_kernel_spmd

F32 = mybir.dt.float32
F32R = mybir.dt.float32r  # fp32 fast-path: 1 PE cycle/row at N>=256 (vs 4 for fp32)
BF16 = mybir.dt.bfloat16

N_CORES = 8
N = 10000          # x rows (softmax width)
NA = 10000         # anchors
D_IN = 256
D_OUT = 64
TEMP = 0.07

NJ_TILES = 79                 # ceil(10000 / 128)
NJ = NJ_TILES * 128           # 10112, padded x-rows
A_CORE = 1280                 # anchors per core (10240 padded / 8)
# anchor-column chunks per pass: sized so every matmul is >=256 wide
# (f32r full rate) and PSUM stays within 8 banks
A_CHUNKS = ((0, 1024), (1024, 256))
M_AUG = D_OUT + 1             # 65: d_out columns + ones column


def _build_bass():
    # Bacc (not plain Bass): its compile() runs the wait-splitting passes
    # (move_matmul_waits_to_ldweights / generate_event_semaphores) that
    # keep every instruction within the TRN2 1-sync-wait ISA limit.
    nc = bacc.Bacc(
        "TRN2",
        target_bir_lowering=False,
        debug=False,
        num_devices=N_CORES,
    )
    xT = nc.dram_tensor("xT", [D_OUT, NJ], F32R, kind="ExternalInput").ap()
    xaug = nc.dram_tensor(
        "xaug", [128, NJ_TILES * M_AUG], BF16, kind="ExternalInput"
    ).ap()
    anchT = nc.dram_tensor("anchT", [D_IN, A_CORE], F32R, kind="ExternalInput").ap()
    adjT = nc.dram_tensor("adjT", [NJ, A_CORE], BF16, kind="ExternalInput").ap()
    wt = nc.dram_tensor("wt", [D_IN, D_OUT], F32R, kind="ExternalInput").ap()
    wscale = nc.dram_tensor("wscale", [128, 1], F32, kind="ExternalInput").ap()
    ident = nc.dram_tensor("ident", [128, 128], F32, kind="ExternalInput").ap()
    out = nc.dram_tensor("out", [A_CORE, D_OUT], F32, kind="ExternalOutput").ap()

    EXP = mybir.ActivationFunctionType.Exp

    with tile.TileContext(nc) as tc:
        with tc.tile_pool(name="const", bufs=1) as const:
            xT_sb = const.tile([D_OUT, NJ], F32R)
            nc.sync.dma_start(xT_sb[:], xT[:])
            xaug_sb = const.tile([128, NJ_TILES * M_AUG], BF16)
            nc.sync.dma_start(xaug_sb[:], xaug[:])
            ident_sb = const.tile([128, 128], F32)
            nc.sync.dma_start(ident_sb[:], ident[:])
            wscale_sb = const.tile([128, 1], F32)
            nc.sync.dma_start(wscale_sb[:], wscale[:])
            qt_sb = const.tile([D_OUT, A_CORE], F32R)
            ot_sb = const.tile([M_AUG, A_CORE], F32)

            # ---- Q^T = wt.T @ anchor.T  -> [64, 1280] ----
            with (
                tc.tile_pool(name="pre", bufs=1) as pre,
                tc.tile_pool(name="prepsum", bufs=1, space="PSUM") as prepsum,
            ):
                an0 = pre.tile([128, A_CORE], F32R)
                nc.sync.dma_start(an0[:], anchT[0:128, :])
                an1 = pre.tile([128, A_CORE], F32R)
                nc.sync.dma_start(an1[:], anchT[128:256, :])
                wt0 = pre.tile([128, D_OUT], F32R)
                nc.sync.dma_start(wt0[:], wt[0:128, :])
                wt1 = pre.tile([128, D_OUT], F32R)
                nc.sync.dma_start(wt1[:], wt[128:256, :])
                qt_ps = prepsum.tile([D_OUT, A_CORE], F32)
                for n0 in range(0, A_CORE, 512):
                    nw = min(512, A_CORE - n0)
                    nc.tensor.matmul(
                        qt_ps[:, n0 : n0 + nw],
                        wt0[:],
                        an0[:, n0 : n0 + nw],
                        start=True,
                        stop=False,
                    )
                    nc.tensor.matmul(
                        qt_ps[:, n0 : n0 + nw],
                        wt1[:],
                        an1[:, n0 : n0 + nw],
                        start=False,
                        stop=True,
                    )
                nc.vector.tensor_copy(qt_sb[:], qt_ps[:])

            # ---- main loop over anchor chunks / j tiles ----
            with (
                tc.tile_pool(name="adjp", bufs=8) as adjp,
                tc.tile_pool(name="pp", bufs=4) as pp,
                tc.tile_pool(name="pmp", bufs=4) as pmp,
                tc.tile_pool(name="spsum", bufs=3, space="PSUM") as spsum,
                tc.tile_pool(name="opsum", bufs=1, space="PSUM") as opsum,
            ):
                for a0, aw in A_CHUNKS:
                    nsplits = [
                        (n0, min(512, aw - n0)) for n0 in range(0, aw, 512)
                    ]
                    o_ps = opsum.tile([M_AUG, aw], F32, padded_shape=[M_AUG, 1024])
                    for j in range(NJ_TILES):
                        xt_w = xT_sb[:, j * 128 : (j + 1) * 128]
                        s_ps = spsum.tile([128, aw], F32, padded_shape=[128, 1024])
                        for n0, nw in nsplits:
                            nc.tensor.matmul(
                                s_ps[:, n0 : n0 + nw],
                                xt_w,
                                qt_sb[:, a0 + n0 : a0 + n0 + nw],
                                start=True,
                                stop=True,
                            )
                        adj_t = adjp.tile([128, aw], BF16, padded_shape=[128, 1024])
                        nc.sync.dma_start(
                            adj_t[:], adjT[j * 128 : (j + 1) * 128, a0 : a0 + aw]
                        )
                        p_t = pp.tile([128, aw], BF16, padded_shape=[128, 1024])
                        nc.scalar.activation(p_t[:], s_ps[:], EXP, scale=1.0 / TEMP)
                        pm_t = pmp.tile([128, aw], BF16, padded_shape=[128, 1024])
                        nc.vector.tensor_mul(pm_t[:], p_t[:], adj_t[:])
                        xa_w = xaug_sb[:, j * M_AUG : (j + 1) * M_AUG]
                        for n0, nw in nsplits:
                            nc.tensor.matmul(
                                o_ps[:, n0 : n0 + nw],
                                xa_w,
                                pm_t[:, n0 : n0 + nw],
                                start=(j == 0),
                                stop=(j == NJ_TILES - 1),
                            )
                    nc.vector.tensor_copy(ot_sb[:, a0 : a0 + aw], o_ps[:])

            # ---- tail: transpose back, normalize, scale, store ----
            with (
                tc.tile_pool(name="tpsum", bufs=2, space="PSUM") as tpsum,
                tc.tile_pool(name="tail", bufs=2) as tail,
            ):
                for k in range(A_CORE // 128):
                    t_ps = tpsum.tile([128, M_AUG], F32)
                    nc.tensor.transpose(
                        t_ps[:],
                        ot_sb[0:M_AUG, k * 128 : (k + 1) * 128],
                        ident_sb[0:M_AUG, 0:M_AUG],
                    )
                    rec = tail.tile([128, 1], F32)
                    nc.vector.reciprocal(rec[:], t_ps[:, D_OUT : D_OUT + 1])
                    rec2 = tail.tile([128, 1], F32)
                    nc.vector.tensor_mul(rec2[:], rec[:], wscale_sb[:])
                    o_t = tail.tile([128, D_OUT], F32)
                    nc.vector.tensor_scalar_mul(o_t[:], t_ps[:, 0:D_OUT], rec2[:])
                    nc.sync.dma_start(out[k * 128 : (k + 1) * 128, :], o_t[:])

    nc.compile()
    return nc


def _prep_inputs(x, weight, adjs, idx, anchor, wt):
    i = int(np.asarray(idx))
    x = np.asarray(x, dtype=np.float32)
    anchor = np.asarray(anchor, dtype=np.float32)
    wt = np.asarray(wt, dtype=np.float32)
    adj = np.asarray(adjs)[i]  # [Na, N] bool
    w = float(np.asarray(weight)[i])

    NAP = N_CORES * A_CORE  # 10240

    xT = np.zeros((D_OUT, NJ), dtype=np.float32)
    xT[:, :N] = x.T

    xaug = np.zeros((NJ, M_AUG), dtype=ml_dtypes.bfloat16)
    xaug[:N, :D_OUT] = x
    xaug[:N, D_OUT] = 1.0
    xaug_strip = np.ascontiguousarray(
        xaug.reshape(NJ_TILES, 128, M_AUG).transpose(1, 0, 2).reshape(128, -1)
    )

    anchorT = np.zeros((D_IN, NAP), dtype=np.float32)
    anchorT[:, :NA] = anchor.T

    # adjacency, transposed to [N, Na], as bf16 {0.0, 1.0}
    adj_u16 = np.zeros((NJ, NAP), dtype=np.uint16)
    adj_u16[:N, :NA] = adj.T
    adj_u16 *= 0x3F80  # bf16 bit pattern of 1.0
    # padded anchor columns: one fake edge to x-row 0 so denominators are
    # finite (those rows are discarded on the host)
    adj_u16[0, NA:] = 0x3F80
    adj_bf = adj_u16.view(ml_dtypes.bfloat16)

    ident = np.eye(128, dtype=np.float32)
    wscale = np.full((128, 1), w, dtype=np.float32)

    in_maps = []
    for c in range(N_CORES):
        sl = slice(c * A_CORE, (c + 1) * A_CORE)
        in_maps.append(
            {
                "xT": xT,
                "xaug": xaug_strip,
                "anchT": np.ascontiguousarray(anchorT[:, sl]),
                "adjT": np.ascontiguousarray(adj_bf[:, sl]),
                "wt": wt,
                "wscale": wscale,
                "ident": ident,
            }
        )
    return in_maps


def run(x, weight, adjs, idx, anchor, wt, trace=False, **spmd_kwargs):
    in_maps = _prep_inputs(x, weight, adjs, idx, anchor, wt)
    nc = _build_bass()
    res = run_bass_kernel_spmd(
        nc, in_maps, core_ids=list(range(N_CORES)), trace=trace, **spmd_kwargs
    )
    out = np.concatenate([r["out"] for r in res.results], axis=0)[:NA]
    return np.ascontiguousarray(out.astype(np.float32)), res


def kernel(x, weight, adjs, idx, anchor, wt):
    out, _ = run(x, weight, adjs, idx, anchor, wt)
    return out


# revision 21
# speedup vs baseline: 1.9227x; 1.0026x over previous
"""Trainium2 Bass kernel: masked (sparse-adjacency) attention.

Computes, for full inputs:
    adj    = adjs[idx]                      # [Na, N] bool
    scores = (anchor @ wt) @ x.T            # [Na, N]
    atten  = softmax(where(adj, scores, -inf) / T, axis=1)
    out    = weight[idx] * (atten @ x)      # [Na, d_out]

Sharding: anchors (rows of the score matrix) are split across the 8
NeuronCores, 1280 rows per core (Na=10000 padded to 10240). x / wt are
replicated; the adjacency is shipped pre-transposed per shard.

Per-core device pipeline (all j-tiles of 128 x-rows, N=10000 padded to
10112 = 79*128):
  S^T[j,a] = X^T.T @ Q^T          (PE, fp32; Q^T = wt.T @ anchor.T)
  P^T      = exp(S^T / T)         (ACT, PSUM -> SBUF, bf16 out)
  PM^T     = P^T * adjT           (DVE tensor_tensor, bf16 2x mode)
  O^T     += [X | 1].T @ PM^T     (PE accumulating over j; the ones
                                   column yields softmax denominators)
Tail: PE-transpose O^T back to [a, 65], scale rows by
weight[idx] / denom, DMA out.

Masking happens *after* exp as a multiply by {0,1}: exp(s/T) is computed
for every entry (bounded: |s| < ~2 for this data, exp arg < ~30), and
multiplying by the adjacency bit zeroes masked entries exactly, which is
mathematically identical to softmax over the masked entries.
"""

import numpy as np
import ml_dtypes

import concourse.bacc as bacc
import concourse.bass as bass
import concourse.mybir as mybir
import concourse.tile as tile
from concourse.bass_utils import run_bass_kernel_spmd

F32 = mybir.dt.float32
F32R = mybir.dt.float32r  # fp32 fast-path: 1 PE cycle/row at N>=256 (vs 4 for fp32)
BF16 = mybir.dt.bfloat16

N_CORES = 8
N = 10000          # x rows (softmax width)
NA = 10000         # anchors
D_IN = 256
D_OUT = 64
TEMP = 0.07

NJ_TILES = 79                 # ceil(10000 / 128)
NJ = NJ_TILES * 128           # 10112, padded x-rows
A_CORE = 1280                 # anchors per core (10240 padded / 8)
# anchor-column chunks per pass: sized so every matmul is >=256 wide
# (f32r full rate) and PSUM stays within 8 banks
A_CHUNKS = ((0, 1024), (1024, 256))
M_AUG = D_OUT + 1             # 65: d_out columns + ones column


def _build_bass():
    # Bacc (not plain Bass): its compile() runs the wait-splitting passes
    # (move_matmul_waits_to_ldweights / generate_event_semaphores) that
    # keep every instruction within the TRN2 1-sync-wait ISA limit.
    nc = bacc.Bacc(
        "TRN2",
        target_bir_lowering=False,
        debug=False,
        num_devices=N_CORES,
    )
    xT = nc.dram_tensor("xT", [D_OUT, NJ], F32R, kind="ExternalInput").ap()
    xaug = nc.dram_tensor(
        "xaug", [128, NJ_TILES * M_AUG], BF16, kind="ExternalInput"
    ).ap()
    anchT = nc.dram_tensor("anchT", [D_IN, A_CORE], F32R, kind="ExternalInput").ap()
    adjT = nc.dram_tensor("adjT", [NJ, A_CORE], BF16, kind="ExternalInput").ap()
    wt = nc.dram_tensor("wt", [D_IN, D_OUT], F32R, kind="ExternalInput").ap()
    wscale = nc.dram_tensor("wscale", [128, 1], F32, kind="ExternalInput").ap()
    ident = nc.dram_tensor("ident", [128, 128], F32, kind="ExternalInput").ap()
    out = nc.dram_tensor("out", [A_CORE, D_OUT], F32, kind="ExternalOutput").ap()

    EXP = mybir.ActivationFunctionType.Exp

    with tile.TileContext(nc) as tc:
        with tc.tile_pool(name="const", bufs=1) as const:
            xT_sb = const.tile([D_OUT, NJ], F32R)
            nc.sync.dma_start(xT_sb[:], xT[:])
            xaug_sb = const.tile([128, NJ_TILES * M_AUG], BF16)
            nc.sync.dma_start(xaug_sb[:], xaug[:])
            ident_sb = const.tile([128, 128], F32)
            nc.sync.dma_start(ident_sb[:], ident[:])
            wscale_sb = const.tile([128, 1], F32)
            nc.sync.dma_start(wscale_sb[:], wscale[:])
            qt_sb = const.tile([D_OUT, A_CORE], F32R)
            ot_sb = const.tile([M_AUG, A_CORE], F32)

            # ---- Q^T = wt.T @ anchor.T  -> [64, 1280] ----
            with (
                tc.tile_pool(name="pre", bufs=1) as pre,
                tc.tile_pool(name="prepsum", bufs=1, space="PSUM") as prepsum,
            ):
                an0 = pre.tile([128, A_CORE], F32R)
                nc.sync.dma_start(an0[:], anchT[0:128, :])
                an1 = pre.tile([128, A_CORE], F32R)
                nc.sync.dma_start(an1[:], anchT[128:256, :])
                wt0 = pre.tile([128, D_OUT], F32R)
                nc.sync.dma_start(wt0[:], wt[0:128, :])
                wt1 = pre.tile([128, D_OUT], F32R)
                nc.sync.dma_start(wt1[:], wt[128:256, :])
                # PE warmup burst: ~7us of dense back-to-back matmul
                # columns so the HAM clock-gate releases (1.2 -> 2.4 GHz)
                # before the main loop; steady-state inter-matmul gaps are
                # far below the ~3.4us re-throttle window, so one burst is
                # enough.
                warm_ps = prepsum.tile([128, 512], F32)
                for _ in range(12):
                    nc.tensor.matmul(
                        warm_ps[:],
                        xT_sb[:, 0:128],
                        xT_sb[:, 0:512],
                        start=True,
                        stop=True,
                    )
                qt_ps = prepsum.tile([D_OUT, A_CORE], F32)
                for n0 in range(0, A_CORE, 512):
                    nw = min(512, A_CORE - n0)
                    nc.tensor.matmul(
                        qt_ps[:, n0 : n0 + nw],
                        wt0[:],
                        an0[:, n0 : n0 + nw],
                        start=True,
                        stop=False,
                    )
                    nc.tensor.matmul(
                        qt_ps[:, n0 : n0 + nw],
                        wt1[:],
                        an1[:, n0 : n0 + nw],
                        start=False,
                        stop=True,
                    )
                nc.vector.tensor_copy(qt_sb[:], qt_ps[:])

            # ---- main loop over anchor chunks / j tiles ----
            with (
                tc.tile_pool(name="adjp", bufs=8) as adjp,
                tc.tile_pool(name="pp", bufs=4) as pp,
                tc.tile_pool(name="pmp", bufs=4) as pmp,
                tc.tile_pool(name="spsum", bufs=3, space="PSUM") as spsum,
                tc.tile_pool(name="opsum", bufs=1, space="PSUM") as opsum,
            ):
                for a0, aw in A_CHUNKS:
                    nsplits = [
                        (n0, min(512, aw - n0)) for n0 in range(0, aw, 512)
                    ]
                    o_ps = opsum.tile([M_AUG, aw], F32, padded_shape=[M_AUG, 1024])
                    for j in range(NJ_TILES):
                        xt_w = xT_sb[:, j * 128 : (j + 1) * 128]
                        s_ps = spsum.tile([128, aw], F32, padded_shape=[128, 1024])
                        for n0, nw in nsplits:
                            nc.tensor.matmul(
                                s_ps[:, n0 : n0 + nw],
                                xt_w,
                                qt_sb[:, a0 + n0 : a0 + n0 + nw],
                                start=True,
                                stop=True,
                            )
                        adj_t = adjp.tile([128, aw], BF16, padded_shape=[128, 1024])
                        nc.sync.dma_start(
                            adj_t[:], adjT[j * 128 : (j + 1) * 128, a0 : a0 + aw]
                        )
                        p_t = pp.tile([128, aw], BF16, padded_shape=[128, 1024])
                        nc.scalar.activation(p_t[:], s_ps[:], EXP, scale=1.0 / TEMP)
                        pm_t = pmp.tile([128, aw], BF16, padded_shape=[128, 1024])
                        nc.vector.tensor_mul(pm_t[:], p_t[:], adj_t[:])
                        xa_w = xaug_sb[:, j * M_AUG : (j + 1) * M_AUG]
                        for n0, nw in nsplits:
                            nc.tensor.matmul(
                                o_ps[:, n0 : n0 + nw],
                                xa_w,
                                pm_t[:, n0 : n0 + nw],
                                start=(j == 0),
                                stop=(j == NJ_TILES - 1),
                            )
                    nc.vector.tensor_copy(ot_sb[:, a0 : a0 + aw], o_ps[:])

            # ---- tail: transpose back, normalize, scale, store ----
            with (
                tc.tile_pool(name="tpsum", bufs=2, space="PSUM") as tpsum,
                tc.tile_pool(name="tail", bufs=2) as tail,
            ):
                for k in range(A_CORE // 128):
                    t_ps = tpsum.tile([128, M_AUG], F32)
                    nc.tensor.transpose(
                        t_ps[:],
                        ot_sb[0:M_AUG, k * 128 : (k + 1) * 128],
                        ident_sb[0:M_AUG, 0:M_AUG],
                    )
                    rec = tail.tile([128, 1], F32)
                    nc.vector.reciprocal(rec[:], t_ps[:, D_OUT : D_OUT + 1])
                    rec2 = tail.tile([128, 1], F32)
                    nc.vector.tensor_mul(rec2[:], rec[:], wscale_sb[:])
                    o_t = tail.tile([128, D_OUT], F32)
                    nc.vector.tensor_scalar_mul(o_t[:], t_ps[:, 0:D_OUT], rec2[:])
                    nc.sync.dma_start(out[k * 128 : (k + 1) * 128, :], o_t[:])

    nc.compile()
    return nc


def _prep_inputs(x, weight, adjs, idx, anchor, wt):
    i = int(np.asarray(idx))
    x = np.asarray(x, dtype=np.float32)
    anchor = np.asarray(anchor, dtype=np.float32)
    wt = np.asarray(wt, dtype=np.float32)
    adj = np.asarray(adjs)[i]  # [Na, N] bool
    w = float(np.asarray(weight)[i])

    NAP = N_CORES * A_CORE  # 10240

    xT = np.zeros((D_OUT, NJ), dtype=np.float32)
    xT[:, :N] = x.T

    xaug = np.zeros((NJ, M_AUG), dtype=ml_dtypes.bfloat16)
    xaug[:N, :D_OUT] = x
    xaug[:N, D_OUT] = 1.0
    xaug_strip = np.ascontiguousarray(
        xaug.reshape(NJ_TILES, 128, M_AUG).transpose(1, 0, 2).reshape(128, -1)
    )

    anchorT = np.zeros((D_IN, NAP), dtype=np.float32)
    anchorT[:, :NA] = anchor.T

    # adjacency, transposed to [N, Na], as bf16 {0.0, 1.0}
    adj_u16 = np.zeros((NJ, NAP), dtype=np.uint16)
    adj_u16[:N, :NA] = adj.T
    adj_u16 *= 0x3F80  # bf16 bit pattern of 1.0
    # padded anchor columns: one fake edge to x-row 0 so denominators are
    # finite (those rows are discarded on the host)
    adj_u16[0, NA:] = 0x3F80
    adj_bf = adj_u16.view(ml_dtypes.bfloat16)

    ident = np.eye(128, dtype=np.float32)
    wscale = np.full((128, 1), w, dtype=np.float32)

    in_maps = []
    for c in range(N_CORES):
        sl = slice(c * A_CORE, (c + 1) * A_CORE)
        in_maps.append(
            {
                "xT": xT,
                "xaug": xaug_strip,
                "anchT": np.ascontiguousarray(anchorT[:, sl]),
                "adjT": np.ascontiguousarray(adj_bf[:, sl]),
                "wt": wt,
                "wscale": wscale,
                "ident": ident,
            }
        )
    return in_maps


def run(x, weight, adjs, idx, anchor, wt, trace=False, **spmd_kwargs):
    in_maps = _prep_inputs(x, weight, adjs, idx, anchor, wt)
    nc = _build_bass()
    res = run_bass_kernel_spmd(
        nc, in_maps, core_ids=list(range(N_CORES)), trace=trace, **spmd_kwargs
    )
    out = np.concatenate([r["out"] for r in res.results], axis=0)[:NA]
    return np.ascontiguousarray(out.astype(np.float32)), res


def kernel(x, weight, adjs, idx, anchor, wt):
    out, _ = run(x, weight, adjs, idx, anchor, wt)
    return out
